# revision 1
# baseline (speedup 1.0000x reference)
"""2-layer GCN (GCNConv 1024->128->3, shared graph) on 8 trn2 NeuronCores.

Strategy (node-sharded, dst-partitioned edges, both layers):
  h~ = dinv * (features @ W1)        -- sharded matmul, per-core 12544 rows
  AllGather h~  -> full table on every core
  layer1: per 128-dst block: gather h~[src] rows (indirect DMA, 128 rows/instr),
          route to dst rows via iota==dstrel one-hot matmul into PSUM,
          x1~ = dinv^2 * relu(agg)   -- table for layer 2
  AllGather x1~
  layer2: same aggregation, then out = dinv * (agg2 @ W2)
Norm factorization: norm_e = dinv[src]*dinv[dst]; dinv[src] folded into the
table rows, dinv[dst] applied post-aggregation (relu commutes: dinv>0).
Self-loops are plain (d,d) edges. Biases are zero in this model (asserted).
Two TileContext sections with nc.reset() between passes keep per-lane DMA
semaphore wait values under the 16-bit ISA limit.
"""
import numpy as np

N_NODES = 100000
IN_CH = 1024
FEAT_CH = 128
OUT_CH = 3
NCORES = 8
P = 128
SHARD = 12544            # 98 blocks of 128 per core; 8*12544 = 100352
NBLK = SHARD // P        # 98
NPAD = NCORES * SHARD    # 100352


def _preprocess(edges2):
    src = np.asarray(edges2[0], dtype=np.int64)
    dst = np.asarray(edges2[1], dtype=np.int64)
    deg = np.bincount(dst, minlength=NPAD).astype(np.float64) + 1.0
    dinv = (deg ** -0.5).astype(np.float32)

    loop = np.arange(NPAD, dtype=np.int64)
    src_a = np.concatenate([src, loop])
    dst_a = np.concatenate([dst, loop])
    order = np.argsort(dst_a, kind="stable")
    src_s = src_a[order]
    dst_s = dst_a[order]

    blk = (dst_s // P).astype(np.int64)
    nblocks_total = NPAD // P
    counts = np.bincount(blk, minlength=nblocks_total)
    C_FIX = int((counts.max() + P - 1) // P)

    esrc = np.zeros((nblocks_total, P, C_FIX), dtype=np.int32)   # pad -> row 0
    edst = np.zeros((nblocks_total, P, C_FIX), dtype=np.float32)
    emask = np.zeros((nblocks_total, P, C_FIX), dtype=np.float32)
    starts = np.concatenate([[0], np.cumsum(counts)])
    k_in_blk = np.arange(len(dst_s)) - starts[blk]
    jj = k_in_blk // P
    pp = k_in_blk % P
    esrc[blk, pp, jj] = src_s.astype(np.int32)
    edst[blk, pp, jj] = (dst_s % P).astype(np.float32)
    emask[blk, pp, jj] = 1.0
    return dinv, esrc, edst, emask, C_FIX


_CACHE = {}


def _build(C_FIX):
    from concourse import bass, mybir, bacc
    from concourse.tile import TileContext
    from concourse.masks import make_identity

    if C_FIX in _CACHE:
        return _CACHE[C_FIX]

    nc = bacc.Bacc("TRN2", target_bir_lowering=False, debug=False, num_devices=NCORES)
    dt = mybir.dt

    featT = nc.dram_tensor("featT", [IN_CH, SHARD], dt.float32, kind="ExternalInput")
    w1 = nc.dram_tensor("w1", [IN_CH, FEAT_CH], dt.float32, kind="ExternalInput")
    w2 = nc.dram_tensor("w2", [FEAT_CH, OUT_CH], dt.float32, kind="ExternalInput")
    dinv_c = nc.dram_tensor("dinv_c", [SHARD, 1], dt.float32, kind="ExternalInput")
    dinv2_c = nc.dram_tensor("dinv2_c", [SHARD, 1], dt.float32, kind="ExternalInput")
    esrc_t = nc.dram_tensor("esrc_t", [NBLK, P, C_FIX], dt.int32, kind="ExternalInput")
    edst_t = nc.dram_tensor("edst_t", [NBLK, P, C_FIX], dt.float32, kind="ExternalInput")
    emask_t = nc.dram_tensor("emask_t", [NBLK, P, C_FIX], dt.float32, kind="ExternalInput")
    out_t = nc.dram_tensor("out_t", [NBLK, P, OUT_CH], dt.float32, kind="ExternalOutput")

    cc_in1 = nc.dram_tensor("cc_in1", [SHARD, FEAT_CH], dt.float32, kind="Internal")
    table1 = nc.dram_tensor("table1", [NPAD, FEAT_CH], dt.float32, kind="Internal",
                            addr_space="Shared")
    cc_in2 = nc.dram_tensor("cc_in2", [SHARD, FEAT_CH], dt.float32, kind="Internal")
    table2 = nc.dram_tensor("table2", [NPAD, FEAT_CH], dt.float32, kind="Internal",
                            addr_space="Shared")

    rg = [list(range(NCORES))]
    dram_local_base = nc.local_dram_base
    dram_shared_base = nc.shared_dram_base

    def load_consts(cpool, wpool, want_w1):
        iota_i = cpool.tile([P, P], dt.int32)
        nc.gpsimd.iota(iota_i[:], pattern=[[1, P]], base=0, channel_multiplier=0)
        iota_f = cpool.tile([P, P], dt.float32)
        nc.vector.tensor_copy(out=iota_f[:], in_=iota_i[:])
        w1_sb = None
        if want_w1:
            w1_sb = wpool.tile([P, IN_CH // P, FEAT_CH], dt.float32)
            for k in range(IN_CH // P):
                nc.sync.dma_start(out=w1_sb[:, k, :], in_=w1[k * P:(k + 1) * P, :])
        w2_sb = wpool.tile([P, OUT_CH], dt.float32)
        nc.sync.dma_start(out=w2_sb[:], in_=w2[:, :])
        dinv_sb = wpool.tile([P, NBLK], dt.float32)
        nc.sync.dma_start(out=dinv_sb[:], in_=dinv_c[:, 0].rearrange("(b p) -> p b", p=P))
        dinv2_sb = wpool.tile([P, NBLK], dt.float32)
        nc.sync.dma_start(out=dinv2_sb[:], in_=dinv2_c[:, 0].rearrange("(b p) -> p b", p=P))
        return iota_f, w1_sb, w2_sb, dinv_sb, dinv2_sb

    def agg_block(table, b, iota_f, bpool, gpool, ohpool, psum):
        es = bpool.tile([P, C_FIX], dt.int32, tag="es")
        nc.scalar.dma_start(out=es[:], in_=esrc_t[b, :, :])
        ed = bpool.tile([P, C_FIX], dt.float32, tag="ed")
        nc.scalar.dma_start(out=ed[:], in_=edst_t[b, :, :])
        em = bpool.tile([P, C_FIX], dt.float32, tag="em")
        nc.scalar.dma_start(out=em[:], in_=emask_t[b, :, :])
        acc = psum.tile([P, FEAT_CH], dt.float32, space="PSUM", tag="acc")
        for j in range(C_FIX):
            gat = gpool.tile([P, FEAT_CH], dt.float32, tag="gat")
            nc.gpsimd.indirect_dma_start(
                out=gat[:], out_offset=None, in_=table[:, :],
                in_offset=bass.IndirectOffsetOnAxis(ap=es[:, j:j + 1], axis=0))
            oh = ohpool.tile([P, P], dt.float32, tag="oh")
            nc.vector.tensor_scalar(
                out=oh[:], in0=iota_f[:],
                scalar1=ed[:, j:j + 1], scalar2=em[:, j:j + 1],
                op0=mybir.AluOpType.is_equal, op1=mybir.AluOpType.mult)
            nc.tensor.matmul(out=acc[:], lhsT=oh[:], rhs=gat[:],
                             start=(j == 0), stop=(j == C_FIX - 1))
        return acc

    # ---------------- section 1: mm1 + AG1 + pass1 + AG2 ----------------
    with TileContext(nc) as tc:
        with tc.tile_pool(name="const", bufs=1) as cpool, \
             tc.tile_pool(name="w", bufs=1) as wpool, \
             tc.tile_pool(name="feat", bufs=12) as fpool, \
             tc.tile_pool(name="gat", bufs=8) as gpool, \
             tc.tile_pool(name="oh", bufs=8) as ohpool, \
             tc.tile_pool(name="blkio", bufs=4) as bpool, \
             tc.tile_pool(name="epi", bufs=4) as epool, \
             tc.tile_pool(name="psum", bufs=4, space="PSUM") as psum:

            iota_f, w1_sb, w2_sb, dinv_sb, dinv2_sb = load_consts(cpool, wpool, True)

            for b in range(NBLK):
                hp = psum.tile([P, FEAT_CH], dt.float32, space="PSUM", tag="hp")
                for k in range(IN_CH // P):
                    ft = fpool.tile([P, P], dt.float32, tag="ft")
                    nc.sync.dma_start(out=ft[:], in_=featT[k * P:(k + 1) * P,
                                                          b * P:(b + 1) * P])
                    nc.tensor.matmul(out=hp[:], lhsT=ft[:], rhs=w1_sb[:, k, :],
                                     start=(k == 0), stop=(k == IN_CH // P - 1))
                hs = epool.tile([P, FEAT_CH], dt.float32, tag="hs")
                nc.vector.tensor_scalar(out=hs[:], in0=hp[:],
                                        scalar1=dinv_sb[:, b:b + 1], scalar2=None,
                                        op0=mybir.AluOpType.mult)
                nc.sync.dma_start(out=cc_in1[b * P:(b + 1) * P, :], in_=hs[:])

            nc.gpsimd.collective_compute(
                "AllGather", mybir.AluOpType.bypass,
                ins=[cc_in1[:, :]], outs=[table1[:, :]],
                replica_groups=rg)

            for b in range(NBLK):
                acc = agg_block(table1, b, iota_f, bpool, gpool, ohpool, psum)
                xr = epool.tile([P, FEAT_CH], dt.float32, tag="xr")
                nc.scalar.activation(out=xr[:], in_=acc[:],
                                     func=mybir.ActivationFunctionType.Relu)
                xs = epool.tile([P, FEAT_CH], dt.float32, tag="xs")
                nc.vector.tensor_scalar(out=xs[:], in0=xr[:],
                                        scalar1=dinv2_sb[:, b:b + 1], scalar2=None,
                                        op0=mybir.AluOpType.mult)
                nc.sync.dma_start(out=cc_in2[b * P:(b + 1) * P, :], in_=xs[:])

            nc.gpsimd.collective_compute(
                "AllGather", mybir.AluOpType.bypass,
                ins=[cc_in2[:, :]], outs=[table2[:, :]],
                replica_groups=rg)

    # ---------------- reset sems, keep DRAM ----------------
    nc.reset(previous_local_dram_base=dram_local_base,
             previous_shared_dram_base=dram_shared_base)

    # ---------------- section 2: pass2 + output ----------------
    with TileContext(nc) as tc:
        with tc.tile_pool(name="const2", bufs=1) as cpool, \
             tc.tile_pool(name="w2p", bufs=1) as wpool, \
             tc.tile_pool(name="gat2", bufs=8) as gpool, \
             tc.tile_pool(name="oh2", bufs=8) as ohpool, \
             tc.tile_pool(name="blkio2", bufs=4) as bpool, \
             tc.tile_pool(name="epi2", bufs=4) as epool, \
             tc.tile_pool(name="psumA", bufs=4, space="PSUM") as psum, \
             tc.tile_pool(name="psumB", bufs=2, space="PSUM") as psum2:

            iota_f, _, w2_sb, dinv_sb, dinv2_sb = load_consts(cpool, wpool, False)
            ident = cpool.tile([P, P], dt.float32)
            make_identity(nc, ident[:])

            for b in range(NBLK):
                acc = agg_block(table2, b, iota_f, bpool, gpool, ohpool, psum)
                a_sb = epool.tile([P, FEAT_CH], dt.float32, tag="a_sb")
                nc.scalar.activation(out=a_sb[:], in_=acc[:],
                                     func=mybir.ActivationFunctionType.Copy)
                aT = psum2.tile([P, P], dt.float32, space="PSUM", tag="aT")
                nc.tensor.transpose(out=aT[:], in_=a_sb[:], identity=ident[:])
                aT_sb = epool.tile([P, P], dt.float32, tag="aT_sb")
                nc.vector.tensor_copy(out=aT_sb[:], in_=aT[:])
                o3 = psum2.tile([P, OUT_CH], dt.float32, space="PSUM", tag="o3")
                nc.tensor.matmul(out=o3[:], lhsT=aT_sb[:], rhs=w2_sb[:],
                                 start=True, stop=True)
                o3s = epool.tile([P, OUT_CH], dt.float32, tag="o3s")
                nc.vector.tensor_scalar(out=o3s[:], in0=o3[:],
                                        scalar1=dinv_sb[:, b:b + 1], scalar2=None,
                                        op0=mybir.AluOpType.mult)
                nc.sync.dma_start(out=out_t[b, :, :], in_=o3s[:])

    nc.compile()
    _CACHE[C_FIX] = nc
    return nc


def kernel(features, edges, edges2, edge_features, additional_feature, W1, b1, W2, b2):
    from concourse.bass_utils import run_bass_kernel_spmd

    features = np.asarray(features, dtype=np.float32)
    edges2 = np.asarray(edges2)
    W1 = np.asarray(W1, dtype=np.float32)
    W2 = np.asarray(W2, dtype=np.float32)
    assert not np.any(np.asarray(b1)) and not np.any(np.asarray(b2)), \
        "nonzero biases not supported by this kernel build"

    dinv, esrc, edst, emask, C_FIX = _preprocess(edges2)

    featT = np.zeros((IN_CH, NPAD), dtype=np.float32)
    featT[:, :N_NODES] = features.T
    dinv2 = dinv * dinv

    nc = _build(C_FIX)

    in_maps = []
    for c in range(NCORES):
        sl = slice(c * SHARD, (c + 1) * SHARD)
        blksl = slice(c * NBLK, (c + 1) * NBLK)
        in_maps.append(dict(
            featT=np.ascontiguousarray(featT[:, sl]),
            w1=W1, w2=W2,
            dinv_c=dinv[sl, None],
            dinv2_c=dinv2[sl, None],
            esrc_t=np.ascontiguousarray(esrc[blksl]),
            edst_t=np.ascontiguousarray(edst[blksl]),
            emask_t=np.ascontiguousarray(emask[blksl]),
        ))

    res = run_bass_kernel_spmd(nc, in_maps, core_ids=list(range(NCORES)))
    out = np.concatenate([r["out_t"].reshape(SHARD, OUT_CH) for r in res.results], axis=0)
    return np.ascontiguousarray(out[:N_NODES]).astype(np.float32)

